# revision 2
# baseline (speedup 1.0000x reference)
"""MoE top-1 routing kernel for Trainium2 (8 NeuronCores, expert-parallel).

Problem: x [N=8192, D=2048] f32, indices [N,1] int (expert id in [0,8)),
W [E=8, D, H=2048] f32, b [E, H] f32.
Output: tokens sorted (stably) by expert id, each row = relu(x @ W[e] + b[e]).

Sharding: expert-parallel, one expert per core. The host routes tokens
(stable argsort by expert id == the required output order), pads each
expert's token set to a common capacity C, and ships x^T segments so the
device computes y^T = relu(W[e]^T @ x^T + b[e]) with W stationary in SBUF.
"""

import math

import numpy as np

import concourse.bass as bass
import concourse.mybir as mybir
import concourse.tile as tile
from concourse import bacc
from concourse.bass_utils import run_bass_kernel_spmd

P = 128           # SBUF partitions
D = 2048          # input features (contraction dim)
H = 2048          # output features
E = 8             # experts == cores
NT = 256          # token chunk (matmul moving free dim; >=256 keeps f32r at 1 cyc/row)

_PROGRAM_CACHE: dict[int, bass.Bass] = {}


def _build_program(C: int) -> bass.Bass:
    """One-core SPMD program: yT[H, C] = relu(W[D,H]^T @ xT[D, C] + b)."""
    assert C % NT == 0
    KT = D // P       # 16 contraction chunks
    MT = H // P       # 16 output-partition chunks
    NCH = C // NT     # token chunks

    nc = bacc.Bacc(None, target_bir_lowering=False, debug=False)

    xT = nc.dram_tensor("xT", [D, C], mybir.dt.float32r, kind="ExternalInput")
    Wd = nc.dram_tensor("W", [D, H], mybir.dt.float32r, kind="ExternalInput")
    bd = nc.dram_tensor("b", [MT, P], mybir.dt.float32, kind="ExternalInput")
    yT = nc.dram_tensor("yT", [H, C], mybir.dt.float32, kind="ExternalOutput")

    with tile.TileContext(nc) as tc:
        with (
            tc.tile_pool(name="wpool", bufs=1) as wpool,
            tc.tile_pool(name="xpool", bufs=2) as xpool,
            tc.tile_pool(name="opool", bufs=3) as opool,
            tc.tile_pool(name="bpool", bufs=1) as bpool,
            tc.tile_pool(name="pspool", bufs=4, space="PSUM") as pspool,
        ):
            btile = bpool.tile([P, MT], mybir.dt.float32, name="btile")
            nc.sync.dma_start(btile[:], bd[:].rearrange("m p -> p m"))

            # W resident in SBUF: 16 tiles of [128, 2048] (one per k chunk)
            wt = []
            for k in range(KT):
                wk = wpool.tile([P, H], mybir.dt.float32r, name=f"w{k}", tag=f"w{k}")
                nc.sync.dma_start(wk[:], Wd[k * P:(k + 1) * P, :])
                wt.append(wk)

            for n in range(NCH):
                xt = []
                for k in range(KT):
                    xk = xpool.tile([P, NT], mybir.dt.float32r,
                                    name=f"x{k}", tag=f"x{k}")
                    nc.sync.dma_start(
                        xk[:], xT[k * P:(k + 1) * P, n * NT:(n + 1) * NT])
                    xt.append(xk)
                for m in range(MT):
                    ps = pspool.tile([P, NT], mybir.dt.float32,
                                     name="ps", tag="ps")
                    for k in range(KT):
                        nc.tensor.matmul(
                            ps[:],
                            wt[k][:, m * P:(m + 1) * P],   # stationary [K=128, M=128]
                            xt[k][:],                      # moving     [K=128, NT]
                            start=(k == 0),
                            stop=(k == KT - 1),
                        )
                    ot = opool.tile([P, NT], mybir.dt.float32, name="ot", tag="ot")
                    nc.scalar.activation(
                        ot[:], ps[:],
                        mybir.ActivationFunctionType.Relu,
                        bias=btile[:, m:m + 1],
                    )
                    nc.sync.dma_start(
                        yT[m * P:(m + 1) * P, n * NT:(n + 1) * NT], ot[:])
    nc.compile()
    return nc


def _get_program(C: int) -> bass.Bass:
    if C not in _PROGRAM_CACHE:
        _PROGRAM_CACHE[C] = _build_program(C)
    return _PROGRAM_CACHE[C]


def kernel(x, indices, W, b):
    x = np.asarray(x, dtype=np.float32)
    W = np.asarray(W, dtype=np.float32)
    b = np.asarray(b, dtype=np.float32)
    idx = np.asarray(indices).reshape(-1).astype(np.int64)
    N = x.shape[0]

    order = np.argsort(idx, kind="stable")
    counts = np.bincount(idx, minlength=E)

    C = int(max(NT, math.ceil(counts.max() / NT) * NT))
    nc = _get_program(C)

    in_maps = []
    pos = 0
    for e in range(E):
        cnt = int(counts[e])
        tok = order[pos:pos + cnt]
        pos += cnt
        xT_e = np.zeros((D, C), dtype=np.float32)
        if cnt:
            xT_e[:, :cnt] = x[tok].T
        in_maps.append({
            "xT": xT_e,
            "W": W[e],
            "b": np.ascontiguousarray(b[e].reshape(H // P, P)),
        })

    results = run_bass_kernel_spmd(nc, in_maps, list(range(E))).results

    out = np.empty((N, H), dtype=np.float32)
    pos = 0
    for e in range(E):
        cnt = int(counts[e])
        if cnt:
            out[pos:pos + cnt] = results[e]["yT"][:, :cnt].T
        pos += cnt
    return out
